# revision 5
# baseline (speedup 1.0000x reference)
"""Trainium2 Bass kernel for the autoregressive VAE (3-layer enc/dec LSTM).

v2: 8-way tensor parallelism over the hidden/gate dim (as v1), with the
per-layer h all-gather implemented as a 3-hop SBUF->SBUF remote-DMA
butterfly (XOR slot order) instead of the ncfw collective:

  phase 0: send my h.T slice   [128,  64] to peer me^1 -> slot 1
  phase 1: send slots 0-1      [128, 128] to peer me^2 -> slots 2-3
  phase 2: send slots 0-3      [128, 256] to peer me^4 -> slots 4-7

so slot j on core p holds h of core p^j; weight contraction chunks are
permuted per-core on the host to match.  Per-slot semaphore waits are
attached directly to the consuming matmuls, so the contraction starts
on early slots while later hops are still in flight.

Other changes vs v1:
  - gate row order [i f o g] -> one sigmoid over 384 cols + one tanh
  - mu/logvar heads merged into one [64, 512] psum accumulation
  - std = exp(lv/2) computed as (1+w)/(1-w), w = tanh(lv/4): no ACT
    table-set switch (sigmoid_and_others holds both sigmoid and tanh)
  - output head computed transposed (lgT [256d x 64b]) so the sigmoid
    feedback x_hat.T needs no PE transpose; logits written transposed
    and fixed up on the host
"""

import sys

sys.path.insert(0, "/opt/trn_rl_repo")

import numpy as np
import ml_dtypes

from concourse import bass, tile, mybir, bacc, bass_interp
from concourse.tile_rust import add_dep_helper
from concourse.bass_utils import run_bass_kernel_spmd


class _PreloadedCoreSim(bass_interp.CoreSim):
    """Tile's scheduling-pass CoreSim with the remote-DMA semaphores
    pre-raised: the single-core scheduler never sees remote increments, so
    real arrival waits would deadlock it. Only used during build()."""

    preload_sems: list = []

    def __init__(self, *a, **k):
        super().__init__(*a, **k)
        for num, name in self.preload_sems:
            self.update_semaphore(
                mybir.SyncUpdate(
                    sync_type="semaphore", id=num, ant_name=name,
                    update_mode="sem-add-imm", update_value=1 << 20,
                )
            )

BF16 = ml_dtypes.bfloat16
L, H, D, B, T_FULL = 3, 1024, 256, 64, 128
NC = 8
SL = H // NC          # 128 h-indices per core
G = 4 * SL            # 512 gate rows per core
AF = mybir.ActivationFunctionType
ALU = mybir.AluOpType
GATE_ORDER = [0, 1, 3, 2]   # [i f o g] row-block order

LAY_NAMES = ["e0", "e1", "e2", "d0", "d1", "d2"]
N_IN_CHUNKS = {"e0": 4, "e1": 8, "e2": 8, "d0": 2, "d1": 8, "d2": 8}


def _chunked_moving(WT, n_cols):
    """[K, n_cols] -> SBUF layout [128, (K//128)*n_cols] bf16, chunk-major."""
    K = WT.shape[0]
    assert K % 128 == 0
    return (
        WT.reshape(K // 128, 128, n_cols)
        .transpose(1, 0, 2)
        .reshape(128, (K // 128) * n_cols)
        .astype(BF16)
    )


def prepare_core_inputs(inputs, core, T=T_FULL, comm="cc"):
    i = inputs
    if comm == "rdma":
        perm = [core ^ j for j in range(NC)]  # slot j holds h of core^j
    else:
        perm = list(range(NC))  # collective fills slots in rank order
    rows_gate = np.concatenate(
        [g * H + SL * core + np.arange(SL) for g in GATE_ORDER]
    )
    m = {}
    layers = [
        ("e0", i["enc_Wih0"], i["enc_Whh"][0], i["enc_b"][0]),
        ("e1", i["enc_Wih"][0], i["enc_Whh"][1], i["enc_b"][1]),
        ("e2", i["enc_Wih"][1], i["enc_Whh"][2], i["enc_b"][2]),
        ("d0", i["dec_Wih0"], i["dec_Whh"][0], i["dec_b"][0]),
        ("d1", i["dec_Wih"][0], i["dec_Whh"][1], i["dec_b"][1]),
        ("d2", i["dec_Wih"][1], i["dec_Whh"][2], i["dec_b"][2]),
    ]
    for name, Wih, Whh, b in layers:
        Wg = Whh[rows_gate]                       # [G, H]
        chunks = [Wg[:, SL * p : SL * (p + 1)].T for p in perm]   # [128, G] each
        m[f"whh_{name}"] = np.concatenate(chunks, axis=1).astype(BF16)
        Wi = Wih[rows_gate]
        if name in ("e0", "d0"):
            m[f"wih_{name}"] = _chunked_moving(np.ascontiguousarray(Wi.T), G)
        else:
            ich = [Wi[:, SL * p : SL * (p + 1)].T for p in perm]
            m[f"wih_{name}"] = np.concatenate(ich, axis=1).astype(BF16)
        m[f"b_{name}"] = b[rows_gate].astype(BF16).reshape(1, G)

    # merged mu|lv head: moving chunk j = [Wmu_j | Wlv_j], each [128, 256]
    mv = []
    for p in perm:
        mv.append(
            np.concatenate(
                [
                    i["W_mu"][:, SL * p : SL * (p + 1)].T,
                    i["W_logvar"][:, SL * p : SL * (p + 1)].T * 0.125,
                ],
                axis=1,
            )
        )
    m["w_mulv"] = np.concatenate(mv, axis=1).astype(BF16)    # [128, 8*512]
    m["b_mulv"] = np.concatenate(
        [i["b_mu"], i["b_logvar"] * 0.125]
    ).astype(BF16).reshape(1, 2 * D)

    # transposed output head: chunk j = [Wout[0:128, h_j].T | Wout[128:256, h_j].T]
    wo = []
    for p in perm:
        blk = i["W_out"][:, SL * p : SL * (p + 1)]           # [256, 128]
        wo.append(np.concatenate([blk[0:128].T, blk[128:256].T], axis=1))  # [128, 256]
    m["w_outT"] = np.concatenate(wo, axis=1).astype(BF16)    # [128, 8*256]
    m["b_outT"] = i["b_out"].astype(BF16).reshape(1, D)

    # x transposed: xT[t, d%128, 64*(d//128)+b] = x[b, t, d]
    xT = (
        i["x"][:, :T]
        .transpose(1, 2, 0)               # [T, D, B]
        .reshape(T, 2, 128, B)
        .transpose(0, 2, 1, 3)            # [T, 128, 2, B]
        .astype(BF16)
        .reshape(T, 128, 2 * B)
    )
    m["xT"] = np.ascontiguousarray(xT)
    m["eps_f"] = np.ascontiguousarray(i["eps"][:, :T].transpose(1, 0, 2)).astype(
        np.float32
    )
    m["ones"] = np.ones((1, B), BF16)
    m["ident"] = np.eye(128, dtype=BF16)
    return m


def build(T=T_FULL, steps=None, comm="cc"):
    nc = bacc.Bacc("TRN2", target_bir_lowering=False, num_devices=NC)
    f32, bf16 = mybir.dt.float32, mybir.dt.bfloat16
    n_steps = T if steps is None else steps

    di = {}
    for n in LAY_NAMES:
        di[f"wih_{n}"] = nc.dram_tensor(
            f"wih_{n}", [128, N_IN_CHUNKS[n] * G], bf16, kind="ExternalInput"
        )
        di[f"whh_{n}"] = nc.dram_tensor(f"whh_{n}", [128, 8 * G], bf16, kind="ExternalInput")
        di[f"b_{n}"] = nc.dram_tensor(f"b_{n}", [1, G], bf16, kind="ExternalInput")
    di["w_mulv"] = nc.dram_tensor("w_mulv", [128, 8 * 2 * D], bf16, kind="ExternalInput")
    di["b_mulv"] = nc.dram_tensor("b_mulv", [1, 2 * D], bf16, kind="ExternalInput")
    di["w_outT"] = nc.dram_tensor("w_outT", [128, 8 * D], bf16, kind="ExternalInput")
    di["b_outT"] = nc.dram_tensor("b_outT", [1, D], bf16, kind="ExternalInput")
    di["xT"] = nc.dram_tensor("xT", [T, 128, 2 * B], bf16, kind="ExternalInput")
    di["eps_f"] = nc.dram_tensor("eps_f", [T, B, D], f32, kind="ExternalInput")
    di["ones"] = nc.dram_tensor("ones", [1, B], bf16, kind="ExternalInput")
    di["ident"] = nc.dram_tensor("ident", [128, 128], bf16, kind="ExternalInput")

    out_mu = nc.dram_tensor("out_mu", [T, B, D], f32, kind="ExternalOutput")
    out_lv = nc.dram_tensor("out_lv", [T, B, D], f32, kind="ExternalOutput")
    out_z = nc.dram_tensor("out_z", [T, B, D], f32, kind="ExternalOutput")
    out_lg = nc.dram_tensor("out_lg", [T, 128, 2 * B], f32, kind="ExternalOutput")

    rg = [list(range(NC))]
    _orig_coresim = tile.CoreSim
    tile.CoreSim = _PreloadedCoreSim

    with tile.TileContext(nc) as tc:
        with (
            tc.tile_pool(name="wpool", bufs=1) as wp,
            tc.tile_pool(name="state", bufs=1) as st,
            tc.tile_pool(name="xio", bufs=4) as xio,
            tc.tile_pool(name="tmp", bufs=3) as tp,
            tc.tile_pool(name="psg", bufs=2, space="PSUM") as psg,
            tc.tile_pool(name="psh", bufs=1, space="PSUM") as psh,
            tc.tile_pool(name="pslg", bufs=2, space="PSUM") as pslg,
            tc.tile_pool(name="pst", bufs=2, space="PSUM") as pst,
            tc.tile_pool(name="dio", bufs=2, space="DRAM") as dio,
        ):
            # ---- weights -> SBUF ----
            w = {}
            for name in di:
                if name in ("xT", "eps_f"):
                    continue
                tile_ = wp.tile(list(di[name].shape), di[name].dtype, tag=f"w_{name}",
                                name=f"w_{name}")
                nc.sync.dma_start(tile_[:], di[name][:])
                w[name] = tile_
            ident64 = w["ident"][0:64, 0:64]

            # ---- persistent state ----
            c_st, g_h = {}, {}
            for n in LAY_NAMES:
                c_st[n] = st.tile([B, SL], f32, tag=f"c_{n}", name=f"c_{n}")
                nc.vector.memset(c_st[n][:], 0.0)
                g_h[n] = []
                for par in range(2):
                    t_ = st.tile([128, NC * B], bf16, tag=f"gh_{n}{par}",
                                 name=f"gh_{n}{par}")
                    nc.vector.memset(t_[:], 0.0)
                    g_h[n].append(t_)
            xhatT = st.tile([128, 2 * B], bf16, tag="xhatT", name="xhatT")

            semR = None
            # ---- semaphores (remote-DMA protocol) ----
            if comm == "rdma":
                semR = {n: [nc.alloc_semaphore(f"semR_{n}_{ph}") for ph in range(3)]
                        for n in LAY_NAMES}
                sem_local = nc.alloc_semaphore("sem_rdma_local")
                _PreloadedCoreSim.preload_sems = [
                    (s.num, s.name) for n in LAY_NAMES for s in semR[n]
                ] + [(sem_local.num, sem_local.name)]
                clear_ins = []
                for n in LAY_NAMES:
                    for s in semR[n]:
                        clear_ins.append(nc.gpsimd.sem_clear(s).ins)
                clear_ins.append(nc.gpsimd.sem_clear(sem_local).ins)
                # cross-core barrier: the AllGather completes only after all
                # ranks contributed, and each rank contributes only after its
                # sem_clears (forced edges below). Gates the first rdma prep.
                bar_sb = st.tile([128, 16], bf16, tag="bar_sb", name="bar_sb")
                ms = nc.vector.memset(bar_sb[:], 0.0)
                for ci in clear_ins:
                    add_dep_helper(ci, ms.ins, True, "sem clears before barrier")
                bar_ib = dio.tile([128, 16], bf16, tag="bar_ib", name="bar_ib")
                nc.sync.dma_start(bar_ib[:], bar_sb[:])
                bar_ob = dio.tile([NC, 128, 16], bf16, tag="bar_ob", name="bar_ob",
                                  addr_space="Shared")
                barrier_coll = nc.gpsimd.collective_compute(
                    "AllGather", ALU.bypass, replica_groups=rg,
                    ins=[bar_ib.opt()], outs=[bar_ob.opt()],
                )
            prep_count = [0]

            def emit_comm(name, t, send=None):
                """all-gather of h.T into g_h[name][t&1]."""
                gh = g_h[name][t & 1]
                if comm == "cc":
                    ib = dio.tile([128, B], bf16, tag="ib", name="ib")
                    nc.sync.dma_start(ib[:], send[:])
                    ob = dio.tile([NC, 128, B], bf16, tag="ob", name="ob",
                                  addr_space="Shared")
                    nc.gpsimd.collective_compute(
                        "AllGather", ALU.bypass, replica_groups=rg,
                        ins=[ib.opt()], outs=[ob.opt()],
                    )
                    nc.sync.dma_start(
                        gh[:].rearrange("p (s j) -> p s j", s=NC),
                        ob[:].rearrange("s p j -> p s j"),
                    )
                    return
                phases = [
                    (1, gh[:, 0:B], gh[:, B : 2 * B]),
                    (2, gh[:, 0 : 2 * B], gh[:, 2 * B : 4 * B]),
                    (4, gh[:, 0 : 4 * B], gh[:, 4 * B : 8 * B]),
                ]
                for ph, (delta, src, dst) in enumerate(phases):
                    rdests = [None] * 8
                    rdests[delta] = (0, delta)
                    prep = nc.gpsimd.remote_dma_broadcast(
                        dst, src, semR[name][ph], sem_local, rdests=rdests
                    )
                    n_prep = prep_count[0]
                    prep_count[0] += 1
                    if n_prep == 0:
                        add_dep_helper(barrier_coll.ins, prep.ins, True,
                                       "init barrier before first rdma")
                    if n_prep >= 12:
                        prep._wait_ge(sem_local, 16 * (n_prep - 11))
                    trig = nc.gpsimd.trigger_dma(count=1)
                    if ph > 0:
                        trig._wait_ge(semR[name][ph - 1], 2 * (t + 1))

            def slot_wait(sems, j, thr):
                """Semaphore wait gating consumption of slot j (round thr//2-1)."""
                if thr <= 0:
                    return None
                if j == 1:
                    return (sems[0], thr)
                if j == 2:
                    return (sems[1], thr)
                if j == 4:
                    return (sems[2], thr)
                return None

            def emit_gates(name, t, psum, in_stat, in_sems, in_thr):
                """bias + recurrent + input matmuls into psum [B, G]."""
                whh, wih, b = w[f"whh_{name}"], w[f"wih_{name}"], w[f"b_{name}"]
                nc.tensor.matmul(psum[:, :], w["ones"][0:1, 0:B], b[0:1, :],
                                 start=True, stop=False)
                ghl = g_h[name][1 - (t & 1)]
                for j in range(NC):
                    ins = nc.tensor.matmul(
                        psum[:, :], ghl[:, B * j : B * (j + 1)],
                        whh[:, j * G : (j + 1) * G], start=False, stop=False,
                    )
                    ww = slot_wait(semR[name], j, 2 * t) if comm == "rdma" else None
                    if ww:
                        ins._wait_ge(*ww)
                n_in = len(in_stat)
                for j, stat in enumerate(in_stat):
                    ins = nc.tensor.matmul(
                        psum[:, :], stat, wih[:, j * G : (j + 1) * G],
                        start=False, stop=(j == n_in - 1),
                    )
                    if in_sems is not None:
                        ww = slot_wait(in_sems, j, in_thr)
                        if ww:
                            ins._wait_ge(*ww)

            def emit_cell(name, psum):
                """psum gates [i f o g] -> h_bf [64,128] bf16; update c."""
                sig = tp.tile([B, 3 * SL], f32, tag="sig", name="sig")
                nc.scalar.activation(sig[:], psum[:, 0 : 3 * SL], AF.Sigmoid)
                tg = tp.tile([B, SL], f32, tag="tg", name="tg")
                nc.scalar.activation(tg[:], psum[:, 3 * SL : 4 * SL], AF.Tanh)
                t1 = tp.tile([B, SL], f32, tag="t1", name="t1")
                nc.vector.tensor_mul(t1[:], sig[:, 0:SL], tg[:])
                t2 = tp.tile([B, SL], f32, tag="t2", name="t2")
                nc.vector.tensor_mul(t2[:], sig[:, SL : 2 * SL], c_st[name][:])
                nc.vector.tensor_add(c_st[name][:], t1[:], t2[:])
                tc_ = tp.tile([B, SL], f32, tag="tc", name="tc")
                nc.scalar.activation(tc_[:], c_st[name][:], AF.Tanh)
                h_bf = tp.tile([B, SL], bf16, tag="h_bf", name="h_bf")
                nc.vector.tensor_mul(h_bf[:], sig[:, 2 * SL : 3 * SL], tc_[:])
                return h_bf

            def emit_layer(name, t, in_stat, in_sems, in_thr):
                psum = psg.tile([B, G], f32, tag="psg", name="psg")
                emit_gates(name, t, psum, in_stat, in_sems, in_thr)
                h_bf = emit_cell(name, psum)
                gh = g_h[name][t & 1]
                pt = pst.tile([128, B], bf16, tag="pt", name="pt")
                nc.tensor.transpose(pt[:], h_bf[:], ident64)
                if comm == "cc":
                    send = tp.tile([128, B], bf16, tag="send", name="send")
                    nc.vector.tensor_copy(send[:], pt[:])
                    emit_comm(name, t, send)
                else:
                    nc.vector.tensor_copy(gh[:, 0:B], pt[:])
                    emit_comm(name, t)

            # ---- prologue ----
            xT_cur = xio.tile([128, 2 * B], bf16, tag="xT", name="xT")
            nc.sync.dma_start(xT_cur[:], di["xT"][0])
            nc.vector.tensor_scalar_add(xhatT[:], xT_cur[:], -0.5)

            # ================= time loop =================
            for t in range(n_steps):
                par = t & 1
                thr_in = 2 * (t + 1)
                if t + 1 < n_steps:
                    xT_next = xio.tile([128, 2 * B], bf16, tag="xT", name="xT")
                    nc.sync.dma_start(xT_next[:], di["xT"][t + 1])
                else:
                    xT_next = None
                eps_t = xio.tile([B, D], f32, tag="eps", name="eps")
                nc.sync.dma_start(eps_t[:], di["eps_f"][t])

                # ---- encoder ----
                emit_layer("e0", t,
                           [xT_cur[:, 0:B], xT_cur[:, B : 2 * B],
                            xhatT[:, 0:B], xhatT[:, B : 2 * B]], None, 0)
                for prev, name in (("e0", "e1"), ("e1", "e2")):
                    pgh = g_h[prev][par]
                    emit_layer(name, t,
                               [pgh[:, B * j : B * (j + 1)] for j in range(NC)],
                               semR[prev] if comm == "rdma" else None, thr_in)

                # ---- heads: mu | logvar merged ----
                pm = psh.tile([B, 2 * D], f32, tag="psh", name="psh")
                nc.tensor.matmul(pm[:, :], w["ones"][0:1, 0:B], w["b_mulv"][0:1, :],
                                 start=True, stop=False)
                ghe2 = g_h["e2"][par]
                for j in range(NC):
                    ins = nc.tensor.matmul(
                        pm[:, :], ghe2[:, B * j : B * (j + 1)],
                        w["w_mulv"][:, j * 2 * D : (j + 1) * 2 * D],
                        start=False, stop=(j == NC - 1),
                    )
                    if comm == "rdma":
                        ww = slot_wait(semR["e2"], j, thr_in)
                        if ww:
                            ins._wait_ge(*ww)
                mlv = tp.tile([B, 2 * D], f32, tag="mlv", name="mlv")
                nc.vector.tensor_copy(mlv[:], pm[:, :])
                mu_sb = mlv[:, 0:D]
                nc.sync.dma_start(out_mu[t], mu_sb)
                lv_sb = tp.tile([B, D], f32, tag="lv_sb", name="lv_sb")
                nc.vector.tensor_scalar_mul(lv_sb[:], mlv[:, D : 2 * D], 8.0)
                nc.sync.dma_start(out_lv[t], lv_sb[:])

                # std = exp(lv/2) = p(u)^4, u = lv/8 (head weights pre-scaled),
                # p = 1 + u + u^2/2 + u^3/6 + u^4/24 (DVE only: no ACT
                # table-set switch, and DVE divide is not a valid TT op)
                u_ = mlv[:, D : 2 * D]
                h1 = tp.tile([B, D], f32, tag="h1", name="h1")
                nc.vector.tensor_scalar(h1[:], u_, 1.0 / 24, 1.0 / 6, ALU.mult, ALU.add)
                h2 = tp.tile([B, D], f32, tag="h2", name="h2")
                nc.vector.tensor_mul(h2[:], h1[:], u_)
                nc.vector.tensor_scalar_add(h2[:], h2[:], 0.5)
                h3 = tp.tile([B, D], f32, tag="h3", name="h3")
                nc.vector.tensor_mul(h3[:], h2[:], u_)
                nc.vector.tensor_scalar_add(h3[:], h3[:], 1.0)
                h4 = tp.tile([B, D], f32, tag="h4", name="h4")
                nc.vector.tensor_mul(h4[:], h3[:], u_)
                nc.vector.tensor_scalar_add(h4[:], h4[:], 1.0)
                s1 = tp.tile([B, D], f32, tag="s1", name="s1")
                nc.vector.tensor_mul(s1[:], h4[:], h4[:])
                std_t = tp.tile([B, D], f32, tag="std", name="std")
                nc.vector.tensor_mul(std_t[:], s1[:], s1[:])
                tz = tp.tile([B, D], f32, tag="tz", name="tz")
                nc.vector.tensor_mul(tz[:], eps_t[:], std_t[:])
                z_sb = tp.tile([B, D], f32, tag="z_sb", name="z_sb")
                nc.vector.tensor_add(z_sb[:], mu_sb, tz[:])
                nc.sync.dma_start(out_z[t], z_sb[:])
                z_bf = tp.tile([B, D], bf16, tag="z_bf", name="z_bf")
                nc.vector.tensor_copy(z_bf[:], z_sb[:])
                zT = tp.tile([128, 2 * B], bf16, tag="zT", name="zT")
                for cix in range(2):
                    ptz = pst.tile([128, B], bf16, tag="pt", name="pt")
                    nc.tensor.transpose(ptz[:], z_bf[:, cix * 128 : (cix + 1) * 128],
                                        ident64)
                    nc.vector.tensor_copy(zT[:, cix * B : (cix + 1) * B], ptz[:])

                # ---- decoder ----
                emit_layer("d0", t, [zT[:, 0:B], zT[:, B : 2 * B]], None, 0)
                for prev, name in (("d0", "d1"), ("d1", "d2")):
                    pgh = g_h[prev][par]
                    emit_layer(name, t,
                               [pgh[:, B * j : B * (j + 1)] for j in range(NC)],
                               semR[prev] if comm == "rdma" else None, thr_in)

                # ---- output head, transposed: lgT [128, 2*64] ----
                plg = pslg.tile([128, 2 * B], f32, tag="pslg", name="pslg")
                ghd2 = g_h["d2"][par]
                for q in range(2):
                    nc.tensor.matmul(
                        plg[:, q * B : (q + 1) * B],
                        w["b_outT"][0:1, q * 128 : (q + 1) * 128],
                        w["ones"][0:1, 0:B], start=True, stop=False,
                        skip_group_check=True,
                    )
                for j in range(NC):
                    for q in range(2):
                        ins = nc.tensor.matmul(
                            plg[:, q * B : (q + 1) * B],
                            w["w_outT"][:, j * D + q * 128 : j * D + (q + 1) * 128],
                            ghd2[:, B * j : B * (j + 1)],
                            start=False, stop=(j == NC - 1),
                            skip_group_check=True,
                        )
                        if comm == "rdma" and q == 0:
                            ww = slot_wait(semR["d2"], j, thr_in)
                            if ww:
                                ins._wait_ge(*ww)
                lg_sb = tp.tile([128, 2 * B], f32, tag="lg_sb", name="lg_sb")
                nc.vector.tensor_copy(lg_sb[:], plg[:, :])
                nc.sync.dma_start(out_lg[t], lg_sb[:])

                # ---- x_hat.T for t+1 ----
                if t + 1 < n_steps:
                    sigT = tp.tile([128, 2 * B], bf16, tag="sigT", name="sigT")
                    nc.scalar.activation(sigT[:], plg[:, :], AF.Sigmoid)
                    nc.vector.tensor_sub(xhatT[:], xT_next[:], sigT[:])
                    xT_cur = xT_next

    tile.CoreSim = _orig_coresim
    _PreloadedCoreSim.preload_sems = []
    nc.compile()
    return nc


_CACHE = {}


def unshard(res, T=T_FULL):
    """core-0 result map -> (rec, logits, mu, logvar, z) in [B, T, D]."""
    r = res
    sig = lambda v: 1.0 / (1.0 + np.exp(-v))
    tb = lambda a: np.ascontiguousarray(np.swapaxes(a, 0, 1))
    lgT = r["out_lg"]                       # [T, 128, 2*64]
    lg = np.empty((T, B, D), np.float32)
    for c in range(2):
        # logits[t, b, 128*c + p] = lgT[t, p, 64*c + b]
        lg[:, :, 128 * c : 128 * (c + 1)] = lgT[:, :, B * c : B * (c + 1)].transpose(
            0, 2, 1
        )
    logits = tb(lg)
    return (
        sig(logits).astype(np.float32),
        logits,
        tb(r["out_mu"]),
        tb(r["out_lv"]),
        tb(r["out_z"]),
    )


def run(inputs, T=T_FULL, trace=False, comm="cc"):
    key = (T, comm)
    if key not in _CACHE:
        _CACHE[key] = build(T, comm=comm)
    nc = _CACHE[key]
    in_maps = [prepare_core_inputs(inputs, k, T, comm=comm) for k in range(NC)]
    res = run_bass_kernel_spmd(nc, in_maps, core_ids=list(range(NC)), trace=trace)
    return unshard(res.results[0], T)


def kernel(**inputs):
    inputs = {k: np.asarray(v) for k, v in inputs.items()}
    return run(inputs, T=T_FULL)
